# revision 11
# baseline (speedup 1.0000x reference)
"""Multi-head attention (B=4, S=2048, D=1024, H=16, causal) on 8 Trainium2
NeuronCores via Bass/Tile.

Three SPMD launches:
  L1  QKV projections, row-sharded: core c computes (x @ W.T + b)^T for its
      1/8 of the B*S rows, all three projections, output in [outcol, rows]
      (transposed) layout, bf16.
  L2  Attention, head-sharded: core c handles heads {2c, 2c+1} for all
      batches.  Scores are computed transposed (ST = K @ Q^T, [k, q] layout)
      so the softmax sum runs over PSUM partitions via a ones-column appended
      to V in the AV matmul - no on-chip transposes anywhere.  Causal
      structure skips upper-triangular score blocks; the triangular boundary
      is applied post-exp as a cheap 0/1 multiply on the [128,128] boundary
      strip of E plus memsets of fully-masked regions.
  L3  Output projection, row-sharded over the B*S rows.

Matmul operands are bf16 (1 cycle/row on the PE, half the DMA);
accumulation is fp32 in PSUM and the softmax denominators stay fp32.
Host work between launches is limited to reshaping/transposing shards and
the final denominator division (softmax normalization commutes with Wo).
"""

import sys

sys.path.insert(0, "/opt/trn_rl_repo")

import ml_dtypes
import numpy as np

import concourse.bacc as bacc
import concourse.tile as tile
from concourse import mybir
from concourse.bass_utils import run_bass_kernel_spmd

F32 = mybir.dt.float32
BF16 = mybir.dt.bfloat16
NPBF = ml_dtypes.bfloat16
EXP = mybir.ActivationFunctionType.Exp

B, S, D, H, DK = 4, 2048, 1024, 16, 64
NCORES = 8
HPC = H // NCORES          # heads per core (2)
RPC = B * S // NCORES      # rows per core in row-sharded launches (1024)
SCALE = 1.0 / np.sqrt(DK)  # folded into the exp activation
NEG = -1e30

_CACHE = {}


def _build_proj():
    """L1: yT = (x @ W.T + b)^T for q/k/v, row shard of 1024 rows."""
    nc = bacc.Bacc(trn_type="TRN2", target_bir_lowering=False)
    ins, outs = {}, {}
    for p in ("q", "k", "v"):
        ins[p] = (
            nc.dram_tensor(f"x{p}", [D, RPC], BF16, kind="ExternalInput"),
            nc.dram_tensor(f"w{p}", [D, D], BF16, kind="ExternalInput"),
            nc.dram_tensor(f"b{p}", [128, D // 128], F32, kind="ExternalInput"),
        )
        outs[p] = nc.dram_tensor(f"{p}t", [D, RPC], BF16, kind="ExternalOutput")

    KT, OCT, RC = D // 128, D // 128, RPC // 512  # 8 k-tiles, 8 oc-tiles, 2 chunks
    with tile.TileContext(nc) as tc:
        with (
            tc.tile_pool(name="big", bufs=2) as big,
            tc.tile_pool(name="bias", bufs=2) as bias,
            tc.tile_pool(name="outp", bufs=3) as outp,
            tc.tile_pool(name="ps", bufs=2, space="PSUM") as psp,
        ):
            for p in ("q", "k", "v"):
                x_d, w_d, b_d = ins[p]
                xt = big.tile([128, KT, RPC], BF16, tag="xt")
                wt = big.tile([128, KT, D], BF16, tag="wt")
                bt = bias.tile([128, OCT], F32, tag="bt")
                for kt in range(KT):
                    nc.sync.dma_start(out=xt[:, kt, :],
                                      in_=x_d[kt * 128:(kt + 1) * 128, :])
                    nc.sync.dma_start(out=wt[:, kt, :],
                                      in_=w_d[kt * 128:(kt + 1) * 128, :])
                nc.sync.dma_start(out=bt[:], in_=b_d[:])
                for oc in range(OCT):
                    ps = psp.tile([128, RPC], F32, tag="ps")
                    for kt in range(KT):
                        lhs = wt[:, kt, oc * 128:(oc + 1) * 128]
                        for rc in range(RC):
                            nc.tensor.matmul(
                                ps[:, rc * 512:(rc + 1) * 512],
                                lhs,
                                xt[:, kt, rc * 512:(rc + 1) * 512],
                                start=(kt == 0),
                                stop=(kt == KT - 1),
                            )
                    ob = outp.tile([128, RPC], BF16, tag="ob")
                    nc.vector.tensor_scalar_add(ob[:], ps[:], bt[:, oc:oc + 1])
                    nc.sync.dma_start(
                        out=outs[p][oc * 128:(oc + 1) * 128, :], in_=ob[:]
                    )
    nc.compile()
    return nc


def _build_attn(causal):
    """L2: attention for 2 heads x 4 batches.

    qt/kt: [128, B*S] bf16 - head pair stacked on partitions (h0: 0-63,
    h1: 64-127), columns b*S+s.
    vp:    [B, 128, HPC, S//128, DK+1] bf16 - V with a ones column appended
           (vp[b, p, hh, t, c] = V'[b, head hh, k = t*128+p, c]).
    m01:   [128, 128] bf16 - causal 0/1 boundary strip (causal mode);
    maskb: [S//128, 128, S] f32 - additive bias in [k, q] layout (general).
    u:     [B, HPC, DK+1, S] f32 - rows 0-63 unnormalized A^T, row 64 the
           softmax denominator.
    """
    nc = bacc.Bacc(trn_type="TRN2", target_bir_lowering=False)
    qt_d = nc.dram_tensor("qt", [128, B * S], BF16, kind="ExternalInput")
    kt_d = nc.dram_tensor("kt", [128, B * S], BF16, kind="ExternalInput")
    vp_d = nc.dram_tensor("vp", [B, 128, HPC, S // 128, DK + 1], BF16,
                          kind="ExternalInput")
    if causal:
        mk_d = nc.dram_tensor("m01", [128, 128], BF16, kind="ExternalInput")
    else:
        mk_d = nc.dram_tensor("maskb", [S // 128, 128, S], F32,
                              kind="ExternalInput")
    u_d = nc.dram_tensor("u", [B, HPC, DK + 1, S], F32, kind="ExternalOutput")

    NJ = S // 512            # 4 q-chunks per batch
    NT = S // 128            # 16 k-tiles per batch
    with tile.TileContext(nc) as tc:
        with (
            tc.tile_pool(name="qk", bufs=1) as qk,
            tc.tile_pool(name="vpool", bufs=2) as vpool,
            tc.tile_pool(name="epool", bufs=6) as epool,
            tc.tile_pool(name="upool", bufs=3) as upool,
            tc.tile_pool(name="mpool", bufs=2) as mpool,
            tc.tile_pool(name="stp", bufs=2, space="PSUM") as stp,
            tc.tile_pool(name="otp", bufs=2, space="PSUM") as otp,
        ):
            qt = qk.tile([128, B * S], BF16, tag="qt")
            kt = qk.tile([128, B * S], BF16, tag="kt")
            nc.sync.dma_start(out=kt[:, 0:S], in_=kt_d[:, 0:S])
            nc.sync.dma_start(out=qt[:, 0:S], in_=qt_d[:, 0:S])
            vp_cur = vpool.tile([128, HPC, S // 128, DK + 1], BF16, tag="vp",
                                name="vp0")
            nc.sync.dma_start(out=vp_cur[:], in_=vp_d[0])
            for bb in range(1, B):
                bsl = slice(bb * S, (bb + 1) * S)
                nc.sync.dma_start(out=kt[:, bsl], in_=kt_d[:, bsl])
                nc.sync.dma_start(out=qt[:, bsl], in_=qt_d[:, bsl])
            if causal:
                mk = qk.tile([128, 128], BF16, tag="mk")
                nc.sync.dma_start(out=mk[:], in_=mk_d[:])
            # Flat software-pipelined emission: the score matmuls + exp for
            # work item k+1 are emitted before the AV matmuls of item k, so
            # the PE never sits behind an exp-wait at chunk boundaries.
            items = []
            for b in range(B):
                for j in range(NJ):
                    ktiles = range(4 * j + 4) if causal else range(NT)
                    last_i = (4 * j + 3) if causal else (NT - 1)
                    for i in ktiles:
                        items.append((b, j, i, last_i))

            vp_tiles = {0: vp_cur}
            ots_map = {}

            def emit_av(b, j, i, last_i, e):
                if (b, j) not in ots_map:
                    ots_map[(b, j)] = [
                        otp.tile([DK + 1, 512], F32, tag=f"ot{hh}",
                                 name=f"ot{hh}_{b}_{j}") for hh in range(HPC)]
                ots = ots_map[(b, j)]
                for hh in range(HPC):
                    nc.tensor.matmul(
                        ots[hh][:],
                        vp_tiles[b][:, hh, i, :],
                        e[:, hh * 512:(hh + 1) * 512],
                        start=(i == 0),
                        stop=(i == last_i),
                    )
                if i == last_i:
                    for hh in range(HPC):
                        uc = upool.tile([DK + 1, 512], F32, tag=f"us{hh}",
                                        name=f"us{hh}_{b}_{j}")
                        nc.vector.tensor_copy(uc[:], ots[hh][:])
                        nc.sync.dma_start(
                            out=u_d[b, hh, :, j * 512:(j + 1) * 512], in_=uc[:])
                    del ots_map[(b, j)]

            pend = None
            for b, j, i, last_i in items:
                if b + 1 < B and b + 1 not in vp_tiles and (j, i) == (0, 0):
                    nv = vpool.tile([128, HPC, S // 128, DK + 1], BF16,
                                    tag="vp", name=f"vp{b + 1}")
                    nc.sync.dma_start(out=nv[:], in_=vp_d[b + 1])
                    vp_tiles[b + 1] = nv
                qsl = slice(b * S + j * 512, b * S + (j + 1) * 512)
                ksl = slice(b * S + i * 128, b * S + (i + 1) * 128)
                st = stp.tile([128, 1024], F32, tag="st")
                nc.tensor.matmul(st[:, 0:512], kt[0:64, ksl],
                                 qt[0:64, qsl], start=True, stop=True)
                nc.tensor.matmul(st[:, 512:1024], kt[64:128, ksl],
                                 qt[64:128, qsl], start=True, stop=True)
                if not causal:
                    mb = mpool.tile([128, 512], F32, tag="mb")
                    nc.sync.dma_start(
                        out=mb[:], in_=mk_d[i, :, j * 512:(j + 1) * 512])
                    nc.vector.tensor_add(st[:, 0:512], st[:, 0:512], mb[:])
                    nc.vector.tensor_add(st[:, 512:1024], st[:, 512:1024], mb[:])
                e = epool.tile([128, 1024], BF16, tag="e")
                diag = causal and i >= 4 * j
                off = (128 * i - 512 * j) if diag else 0
                if off >= 256:
                    # skip exp over the fully-masked leading columns
                    for hh in range(HPC):
                        o = hh * 512 + off
                        nc.scalar.activation(e[:, o:hh * 512 + 512],
                                             st[:, o:hh * 512 + 512],
                                             EXP, scale=float(SCALE))
                else:
                    nc.scalar.activation(e[:], st[:], EXP, scale=float(SCALE))
                if diag:
                    for hh in range(HPC):
                        o = hh * 512 + off
                        nc.vector.tensor_mul(
                            e[:, o:o + 128], e[:, o:o + 128], mk[:])
                        if off:
                            nc.vector.memset(e[:, hh * 512:hh * 512 + off], 0.0)
                if pend is not None:
                    emit_av(*pend)
                pend = (b, j, i, last_i, e)
            if pend is not None:
                emit_av(*pend)
    nc.compile()
    return nc


def _build_outproj():
    """L3: y = A @ Wo.T for a 1024-row shard (bias added on host)."""
    nc = bacc.Bacc(trn_type="TRN2", target_bir_lowering=False)
    at_d = nc.dram_tensor("at", [D, RPC], BF16, kind="ExternalInput")
    wo_d = nc.dram_tensor("wo", [D, D], BF16, kind="ExternalInput")
    y_d = nc.dram_tensor("y", [RPC, D], F32, kind="ExternalOutput")

    KT, RB = D // 128, RPC // 128
    with tile.TileContext(nc) as tc:
        with (
            tc.tile_pool(name="big", bufs=1) as big,
            tc.tile_pool(name="outp", bufs=3) as outp,
            tc.tile_pool(name="ps", bufs=2, space="PSUM") as psp,
        ):
            at = big.tile([128, KT, RPC], BF16, tag="at")
            wo = big.tile([128, KT, D], BF16, tag="wo")
            for kt in range(KT):
                nc.sync.dma_start(out=at[:, kt, :],
                                  in_=at_d[kt * 128:(kt + 1) * 128, :])
                nc.sync.dma_start(out=wo[:, kt, :],
                                  in_=wo_d[kt * 128:(kt + 1) * 128, :])
            for rb in range(RB):
                ps = psp.tile([128, D], F32, tag="ps")
                for kt in range(KT):
                    lhs = at[:, kt, rb * 128:(rb + 1) * 128]
                    for oc in range(D // 512):
                        nc.tensor.matmul(
                            ps[:, oc * 512:(oc + 1) * 512],
                            lhs,
                            wo[:, kt, oc * 512:(oc + 1) * 512],
                            start=(kt == 0),
                            stop=(kt == KT - 1),
                        )
                ob = outp.tile([128, D], F32, tag="ob")
                nc.vector.tensor_copy(ob[:], ps[:])
                nc.sync.dma_start(out=y_d[rb * 128:(rb + 1) * 128, :], in_=ob[:])
    nc.compile()
    return nc


def _get(name, builder, *args):
    if name not in _CACHE:
        _CACHE[name] = builder(*args)
    return _CACHE[name]


def _strip_mask01():
    # m01[p, g] = 1 where the element (k = p, q = g) of the boundary strip is
    # causally valid (g >= p), else 0.
    p = np.arange(128)[:, None]
    g = np.arange(128)[None, :]
    return (g >= p).astype(NPBF)


def kernel(q, k, v, mask, Wq, bq, Wk, bk, Wv, bv, Wo, bo):
    q = np.asarray(q, dtype=np.float32)
    k = np.asarray(k, dtype=np.float32)
    v = np.asarray(v, dtype=np.float32)
    mask = np.asarray(mask)
    cores = list(range(NCORES))

    # ---------------- L1: QKV projections (row-sharded) ----------------
    nc1 = _get("proj", _build_proj)
    xqT = np.ascontiguousarray(q.reshape(B * S, D).T.astype(NPBF))   # [D, B*S]
    xkT = np.ascontiguousarray(k.reshape(B * S, D).T.astype(NPBF))
    xvT = np.ascontiguousarray(v.reshape(B * S, D).T.astype(NPBF))
    wqT = np.ascontiguousarray(np.asarray(Wq, np.float32).T.astype(NPBF))
    wkT = np.ascontiguousarray(np.asarray(Wk, np.float32).T.astype(NPBF))
    wvT = np.ascontiguousarray(np.asarray(Wv, np.float32).T.astype(NPBF))
    bqt = np.ascontiguousarray(np.asarray(bq, np.float32).reshape(D // 128, 128).T)
    bkt = np.ascontiguousarray(np.asarray(bk, np.float32).reshape(D // 128, 128).T)
    bvt = np.ascontiguousarray(np.asarray(bv, np.float32).reshape(D // 128, 128).T)
    in1 = [
        {
            "xq": np.ascontiguousarray(xqT[:, c * RPC:(c + 1) * RPC]),
            "xk": np.ascontiguousarray(xkT[:, c * RPC:(c + 1) * RPC]),
            "xv": np.ascontiguousarray(xvT[:, c * RPC:(c + 1) * RPC]),
            "wq": wqT, "wk": wkT, "wv": wvT,
            "bq": bqt, "bk": bkt, "bv": bvt,
        }
        for c in cores
    ]
    r1 = run_bass_kernel_spmd(nc1, in1, core_ids=cores)
    QT = np.concatenate([r1.results[c]["qt"] for c in cores], axis=1)  # [D, B*S]
    KTm = np.concatenate([r1.results[c]["kt"] for c in cores], axis=1)
    VT = np.concatenate([r1.results[c]["vt"] for c in cores], axis=1)

    # ---------------- L2: attention (head-sharded) ----------------------
    m2 = mask.reshape(S, S)
    causal = bool(np.array_equal(m2 != 0, np.tril(np.ones((S, S), bool))))
    allones = bool((m2 != 0).all())
    use_causal = causal and not allones
    nc2 = _get(("attn", use_causal), _build_attn, use_causal)

    # V' per core: [B, 128, HPC, S//128, DK+1]
    Vh = VT.reshape(H, DK, B, S)                       # [h, d, b, s]
    in2 = []
    for c in cores:
        vp = np.empty((B, 128, HPC, S // 128, DK + 1), NPBF)
        for hh in range(HPC):
            h = HPC * c + hh
            # [d, b, s] -> [b, s, d] -> [b, t, p, d]
            vb = np.transpose(Vh[h], (1, 2, 0)).reshape(B, S // 128, 128, DK)
            vp[:, :, hh, :, :DK] = np.transpose(vb, (0, 2, 1, 3))
            vp[:, :, hh, :, DK] = 1.0
        m = {
            "qt": np.ascontiguousarray(QT[c * 128:(c + 1) * 128]),
            "kt": np.ascontiguousarray(KTm[c * 128:(c + 1) * 128]),
            "vp": vp,
        }
        if use_causal:
            m["m01"] = _strip_mask01()
        else:
            bias = np.where(m2 != 0, 0.0, NEG).astype(np.float32)
            if allones:
                bias[:] = 0.0
            # biasT[k, q] layout, tiled [S//128, 128, S]
            m["maskb"] = np.ascontiguousarray(bias.T.reshape(S // 128, 128, S))
        in2.append(m)
    r2 = run_bass_kernel_spmd(nc2, in2, core_ids=cores)

    # ---------------- normalize + L3: output projection -----------------
    UA = np.empty((D, B * S), np.float32)  # A^T, normalized
    for c in cores:
        u = r2.results[c]["u"]             # [B, HPC, DK+1, S]
        for hh in range(HPC):
            h = HPC * c + hh
            a = u[:, hh, :DK, :] / u[:, hh, DK:DK + 1, :]   # [B, DK, S]
            UA[h * DK:(h + 1) * DK] = np.transpose(a, (1, 0, 2)).reshape(DK, B * S)

    nc3 = _get("outproj", _build_outproj)
    UAb = UA.astype(NPBF)
    woT = np.ascontiguousarray(np.asarray(Wo, np.float32).T.astype(NPBF))
    in3 = [
        {"at": np.ascontiguousarray(UAb[:, c * RPC:(c + 1) * RPC]), "wo": woT}
        for c in cores
    ]
    r3 = run_bass_kernel_spmd(nc3, in3, core_ids=cores)
    y = np.concatenate([r3.results[c]["y"] for c in cores], axis=0)
    y = y + np.asarray(bo, np.float32)[None, :]
    return y.reshape(B, S, D)


# revision 12
# speedup vs baseline: 1.0448x; 1.0448x over previous
"""Multi-head attention (B=4, S=2048, D=1024, H=16, causal) on 8 Trainium2
NeuronCores via Bass/Tile.

Three SPMD launches:
  L1  QKV projections, row-sharded: core c computes (x @ W.T + b)^T for its
      1/8 of the B*S rows, all three projections, output in [outcol, rows]
      (transposed) layout, bf16.
  L2  Attention, head-sharded: core c handles heads {2c, 2c+1} for all
      batches.  Scores are computed transposed (ST = K @ Q^T, [k, q] layout)
      so the softmax sum runs over PSUM partitions via a ones-column appended
      to V in the AV matmul - no on-chip transposes anywhere.  Causal
      structure skips upper-triangular score blocks; the triangular boundary
      is applied post-exp as a cheap 0/1 multiply on the [128,128] boundary
      strip of E plus memsets of fully-masked regions.
  L3  Output projection, row-sharded over the B*S rows.

Matmul operands are bf16 (1 cycle/row on the PE, half the DMA);
accumulation is fp32 in PSUM and the softmax denominators stay fp32.
Host work between launches is limited to reshaping/transposing shards and
the final denominator division (softmax normalization commutes with Wo).
"""

import sys

sys.path.insert(0, "/opt/trn_rl_repo")

import ml_dtypes
import numpy as np

import concourse.bacc as bacc
import concourse.tile as tile
from concourse import mybir
from concourse.bass_utils import run_bass_kernel_spmd

F32 = mybir.dt.float32
BF16 = mybir.dt.bfloat16
NPBF = ml_dtypes.bfloat16
EXP = mybir.ActivationFunctionType.Exp

B, S, D, H, DK = 4, 2048, 1024, 16, 64
NCORES = 8
HPC = H // NCORES          # heads per core (2)
RPC = B * S // NCORES      # rows per core in row-sharded launches (1024)
SCALE = 1.0 / np.sqrt(DK)  # folded into the exp activation
NEG = -1e30

_CACHE = {}


def _build_proj():
    """L1: yT = (x @ W.T + b)^T for q/k/v, row shard of 1024 rows."""
    nc = bacc.Bacc(trn_type="TRN2", target_bir_lowering=False)
    ins, outs = {}, {}
    for p in ("q", "k", "v"):
        ins[p] = (
            nc.dram_tensor(f"x{p}", [D, RPC], BF16, kind="ExternalInput"),
            nc.dram_tensor(f"w{p}", [D, D], BF16, kind="ExternalInput"),
            nc.dram_tensor(f"b{p}", [128, D // 128], F32, kind="ExternalInput"),
        )
        outs[p] = nc.dram_tensor(f"{p}t", [D, RPC], BF16, kind="ExternalOutput")

    KT, OCT, RC = D // 128, D // 128, RPC // 512  # 8 k-tiles, 8 oc-tiles, 2 chunks
    with tile.TileContext(nc) as tc:
        with (
            tc.tile_pool(name="big", bufs=2) as big,
            tc.tile_pool(name="bias", bufs=2) as bias,
            tc.tile_pool(name="outp", bufs=3) as outp,
            tc.tile_pool(name="ps", bufs=2, space="PSUM") as psp,
        ):
            for p in ("q", "k", "v"):
                x_d, w_d, b_d = ins[p]
                xt = big.tile([128, KT, RPC], BF16, tag="xt")
                wt = big.tile([128, KT, D], BF16, tag="wt")
                bt = bias.tile([128, OCT], F32, tag="bt")
                for kt in range(KT):
                    nc.sync.dma_start(out=xt[:, kt, :],
                                      in_=x_d[kt * 128:(kt + 1) * 128, :])
                    nc.sync.dma_start(out=wt[:, kt, :],
                                      in_=w_d[kt * 128:(kt + 1) * 128, :])
                nc.sync.dma_start(out=bt[:], in_=b_d[:])
                for oc in range(OCT):
                    ps = psp.tile([128, RPC], F32, tag="ps")
                    for kt in range(KT):
                        lhs = wt[:, kt, oc * 128:(oc + 1) * 128]
                        for rc in range(RC):
                            nc.tensor.matmul(
                                ps[:, rc * 512:(rc + 1) * 512],
                                lhs,
                                xt[:, kt, rc * 512:(rc + 1) * 512],
                                start=(kt == 0),
                                stop=(kt == KT - 1),
                            )
                    ob = outp.tile([128, RPC], BF16, tag="ob")
                    nc.vector.tensor_scalar_add(ob[:], ps[:], bt[:, oc:oc + 1])
                    nc.sync.dma_start(
                        out=outs[p][oc * 128:(oc + 1) * 128, :], in_=ob[:]
                    )
    nc.compile()
    return nc


def _build_attn(causal):
    """L2: attention for 2 heads x 4 batches.

    qt/kt: [128, B*S] bf16 - head pair stacked on partitions (h0: 0-63,
    h1: 64-127), columns b*S+s.
    vp:    [B, 128, HPC, S//128, DK+1] bf16 - V with a ones column appended
           (vp[b, p, hh, t, c] = V'[b, head hh, k = t*128+p, c]).
    m01:   [128, 128] bf16 - causal 0/1 boundary strip (causal mode);
    maskb: [S//128, 128, S] f32 - additive bias in [k, q] layout (general).
    u:     [B, HPC, DK+1, S] f32 - rows 0-63 unnormalized A^T, row 64 the
           softmax denominator.
    """
    nc = bacc.Bacc(trn_type="TRN2", target_bir_lowering=False)
    qt_d = nc.dram_tensor("qt", [128, B * S], BF16, kind="ExternalInput")
    kt_d = nc.dram_tensor("kt", [128, B * S], BF16, kind="ExternalInput")
    vp_d = nc.dram_tensor("vp", [B, 128, HPC, S // 128, DK + 1], BF16,
                          kind="ExternalInput")
    if causal:
        mk_d = nc.dram_tensor("m01", [128, 128], BF16, kind="ExternalInput")
    else:
        mk_d = nc.dram_tensor("maskb", [S // 128, 128, S], F32,
                              kind="ExternalInput")
    u_d = nc.dram_tensor("u", [B, HPC, DK + 1, S], F32, kind="ExternalOutput")

    NJ = S // 512            # 4 q-chunks per batch
    NT = S // 128            # 16 k-tiles per batch
    with tile.TileContext(nc) as tc:
        with (
            tc.tile_pool(name="qk", bufs=1) as qk,
            tc.tile_pool(name="vpool", bufs=2) as vpool,
            tc.tile_pool(name="epool", bufs=6) as epool,
            tc.tile_pool(name="upool", bufs=3) as upool,
            tc.tile_pool(name="mpool", bufs=2) as mpool,
            tc.tile_pool(name="stp", bufs=3, space="PSUM") as stp,
            tc.tile_pool(name="otp", bufs=1, space="PSUM") as otp,
        ):
            qt = qk.tile([128, B * S], BF16, tag="qt")
            kt = qk.tile([128, B * S], BF16, tag="kt")
            nc.sync.dma_start(out=kt[:, 0:S], in_=kt_d[:, 0:S])
            nc.sync.dma_start(out=qt[:, 0:S], in_=qt_d[:, 0:S])
            vp_cur = vpool.tile([128, HPC, S // 128, DK + 1], BF16, tag="vp",
                                name="vp0")
            nc.sync.dma_start(out=vp_cur[:], in_=vp_d[0])
            for bb in range(1, B):
                bsl = slice(bb * S, (bb + 1) * S)
                nc.sync.dma_start(out=kt[:, bsl], in_=kt_d[:, bsl])
                nc.sync.dma_start(out=qt[:, bsl], in_=qt_d[:, bsl])
            if causal:
                mk = qk.tile([128, 128], BF16, tag="mk")
                nc.sync.dma_start(out=mk[:], in_=mk_d[:])
            # Flat software-pipelined emission: the score matmuls + exp for
            # work item k+1 are emitted before the AV matmuls of item k, so
            # the PE never sits behind an exp-wait at chunk boundaries.
            items = []
            for b in range(B):
                for j in range(NJ):
                    ktiles = range(4 * j + 4) if causal else range(NT)
                    last_i = (4 * j + 3) if causal else (NT - 1)
                    for i in ktiles:
                        items.append((b, j, i, last_i))

            vp_tiles = {0: vp_cur}
            ots_map = {}

            def emit_av(b, j, i, last_i, e):
                if (b, j) not in ots_map:
                    ots_map[(b, j)] = [
                        otp.tile([DK + 1, 512], F32, tag=f"ot{hh}",
                                 name=f"ot{hh}_{b}_{j}") for hh in range(HPC)]
                ots = ots_map[(b, j)]
                for hh in range(HPC):
                    nc.tensor.matmul(
                        ots[hh][:],
                        vp_tiles[b][:, hh, i, :],
                        e[:, hh * 512:(hh + 1) * 512],
                        start=(i == 0),
                        stop=(i == last_i),
                    )
                if i == last_i:
                    for hh in range(HPC):
                        uc = upool.tile([DK + 1, 512], F32, tag=f"us{hh}",
                                        name=f"us{hh}_{b}_{j}")
                        nc.vector.tensor_copy(uc[:], ots[hh][:])
                        nc.sync.dma_start(
                            out=u_d[b, hh, :, j * 512:(j + 1) * 512], in_=uc[:])
                    del ots_map[(b, j)]

            pend = None
            for b, j, i, last_i in items:
                if b + 1 < B and b + 1 not in vp_tiles and (j, i) == (0, 0):
                    nv = vpool.tile([128, HPC, S // 128, DK + 1], BF16,
                                    tag="vp", name=f"vp{b + 1}")
                    nc.sync.dma_start(out=nv[:], in_=vp_d[b + 1])
                    vp_tiles[b + 1] = nv
                qsl = slice(b * S + j * 512, b * S + (j + 1) * 512)
                ksl = slice(b * S + i * 128, b * S + (i + 1) * 128)
                st = stp.tile([128, 1024], F32, tag="st")
                nc.tensor.matmul(st[:, 0:512], kt[0:64, ksl],
                                 qt[0:64, qsl], start=True, stop=True)
                nc.tensor.matmul(st[:, 512:1024], kt[64:128, ksl],
                                 qt[64:128, qsl], start=True, stop=True)
                if not causal:
                    mb = mpool.tile([128, 512], F32, tag="mb")
                    nc.sync.dma_start(
                        out=mb[:], in_=mk_d[i, :, j * 512:(j + 1) * 512])
                    nc.vector.tensor_add(st[:, 0:512], st[:, 0:512], mb[:])
                    nc.vector.tensor_add(st[:, 512:1024], st[:, 512:1024], mb[:])
                e = epool.tile([128, 1024], BF16, tag="e")
                diag = causal and i >= 4 * j
                off = (128 * i - 512 * j) if diag else 0
                if off >= 256:
                    # skip exp over the fully-masked leading columns
                    for hh in range(HPC):
                        o = hh * 512 + off
                        nc.scalar.activation(e[:, o:hh * 512 + 512],
                                             st[:, o:hh * 512 + 512],
                                             EXP, scale=float(SCALE))
                else:
                    nc.scalar.activation(e[:], st[:], EXP, scale=float(SCALE))
                if diag:
                    for hh in range(HPC):
                        o = hh * 512 + off
                        nc.vector.tensor_mul(
                            e[:, o:o + 128], e[:, o:o + 128], mk[:])
                        if off:
                            nc.vector.memset(e[:, hh * 512:hh * 512 + off], 0.0)
                if pend is not None:
                    emit_av(*pend)
                pend = (b, j, i, last_i, e)
            if pend is not None:
                emit_av(*pend)
    nc.compile()
    return nc


def _build_outproj():
    """L3: y = A @ Wo.T for a 1024-row shard (bias added on host)."""
    nc = bacc.Bacc(trn_type="TRN2", target_bir_lowering=False)
    at_d = nc.dram_tensor("at", [D, RPC], BF16, kind="ExternalInput")
    wo_d = nc.dram_tensor("wo", [D, D], BF16, kind="ExternalInput")
    y_d = nc.dram_tensor("y", [RPC, D], F32, kind="ExternalOutput")

    KT, RB = D // 128, RPC // 128
    with tile.TileContext(nc) as tc:
        with (
            tc.tile_pool(name="big", bufs=1) as big,
            tc.tile_pool(name="outp", bufs=3) as outp,
            tc.tile_pool(name="ps", bufs=2, space="PSUM") as psp,
        ):
            at = big.tile([128, KT, RPC], BF16, tag="at")
            wo = big.tile([128, KT, D], BF16, tag="wo")
            for kt in range(KT):
                nc.sync.dma_start(out=at[:, kt, :],
                                  in_=at_d[kt * 128:(kt + 1) * 128, :])
                nc.sync.dma_start(out=wo[:, kt, :],
                                  in_=wo_d[kt * 128:(kt + 1) * 128, :])
            for rb in range(RB):
                ps = psp.tile([128, D], F32, tag="ps")
                for kt in range(KT):
                    lhs = at[:, kt, rb * 128:(rb + 1) * 128]
                    for oc in range(D // 512):
                        nc.tensor.matmul(
                            ps[:, oc * 512:(oc + 1) * 512],
                            lhs,
                            wo[:, kt, oc * 512:(oc + 1) * 512],
                            start=(kt == 0),
                            stop=(kt == KT - 1),
                        )
                ob = outp.tile([128, D], F32, tag="ob")
                nc.vector.tensor_copy(ob[:], ps[:])
                nc.sync.dma_start(out=y_d[rb * 128:(rb + 1) * 128, :], in_=ob[:])
    nc.compile()
    return nc


def _get(name, builder, *args):
    if name not in _CACHE:
        _CACHE[name] = builder(*args)
    return _CACHE[name]


def _strip_mask01():
    # m01[p, g] = 1 where the element (k = p, q = g) of the boundary strip is
    # causally valid (g >= p), else 0.
    p = np.arange(128)[:, None]
    g = np.arange(128)[None, :]
    return (g >= p).astype(NPBF)


def kernel(q, k, v, mask, Wq, bq, Wk, bk, Wv, bv, Wo, bo):
    q = np.asarray(q, dtype=np.float32)
    k = np.asarray(k, dtype=np.float32)
    v = np.asarray(v, dtype=np.float32)
    mask = np.asarray(mask)
    cores = list(range(NCORES))

    # ---------------- L1: QKV projections (row-sharded) ----------------
    nc1 = _get("proj", _build_proj)
    xqT = np.ascontiguousarray(q.reshape(B * S, D).T.astype(NPBF))   # [D, B*S]
    xkT = np.ascontiguousarray(k.reshape(B * S, D).T.astype(NPBF))
    xvT = np.ascontiguousarray(v.reshape(B * S, D).T.astype(NPBF))
    wqT = np.ascontiguousarray(np.asarray(Wq, np.float32).T.astype(NPBF))
    wkT = np.ascontiguousarray(np.asarray(Wk, np.float32).T.astype(NPBF))
    wvT = np.ascontiguousarray(np.asarray(Wv, np.float32).T.astype(NPBF))
    bqt = np.ascontiguousarray(np.asarray(bq, np.float32).reshape(D // 128, 128).T)
    bkt = np.ascontiguousarray(np.asarray(bk, np.float32).reshape(D // 128, 128).T)
    bvt = np.ascontiguousarray(np.asarray(bv, np.float32).reshape(D // 128, 128).T)
    in1 = [
        {
            "xq": np.ascontiguousarray(xqT[:, c * RPC:(c + 1) * RPC]),
            "xk": np.ascontiguousarray(xkT[:, c * RPC:(c + 1) * RPC]),
            "xv": np.ascontiguousarray(xvT[:, c * RPC:(c + 1) * RPC]),
            "wq": wqT, "wk": wkT, "wv": wvT,
            "bq": bqt, "bk": bkt, "bv": bvt,
        }
        for c in cores
    ]
    r1 = run_bass_kernel_spmd(nc1, in1, core_ids=cores)
    QT = np.concatenate([r1.results[c]["qt"] for c in cores], axis=1)  # [D, B*S]
    KTm = np.concatenate([r1.results[c]["kt"] for c in cores], axis=1)
    VT = np.concatenate([r1.results[c]["vt"] for c in cores], axis=1)

    # ---------------- L2: attention (head-sharded) ----------------------
    m2 = mask.reshape(S, S)
    causal = bool(np.array_equal(m2 != 0, np.tril(np.ones((S, S), bool))))
    allones = bool((m2 != 0).all())
    use_causal = causal and not allones
    nc2 = _get(("attn", use_causal), _build_attn, use_causal)

    # V' per core: [B, 128, HPC, S//128, DK+1]
    Vh = VT.reshape(H, DK, B, S)                       # [h, d, b, s]
    in2 = []
    for c in cores:
        vp = np.empty((B, 128, HPC, S // 128, DK + 1), NPBF)
        for hh in range(HPC):
            h = HPC * c + hh
            # [d, b, s] -> [b, s, d] -> [b, t, p, d]
            vb = np.transpose(Vh[h], (1, 2, 0)).reshape(B, S // 128, 128, DK)
            vp[:, :, hh, :, :DK] = np.transpose(vb, (0, 2, 1, 3))
            vp[:, :, hh, :, DK] = 1.0
        m = {
            "qt": np.ascontiguousarray(QT[c * 128:(c + 1) * 128]),
            "kt": np.ascontiguousarray(KTm[c * 128:(c + 1) * 128]),
            "vp": vp,
        }
        if use_causal:
            m["m01"] = _strip_mask01()
        else:
            bias = np.where(m2 != 0, 0.0, NEG).astype(np.float32)
            if allones:
                bias[:] = 0.0
            # biasT[k, q] layout, tiled [S//128, 128, S]
            m["maskb"] = np.ascontiguousarray(bias.T.reshape(S // 128, 128, S))
        in2.append(m)
    r2 = run_bass_kernel_spmd(nc2, in2, core_ids=cores)

    # ---------------- normalize + L3: output projection -----------------
    UA = np.empty((D, B * S), np.float32)  # A^T, normalized
    for c in cores:
        u = r2.results[c]["u"]             # [B, HPC, DK+1, S]
        for hh in range(HPC):
            h = HPC * c + hh
            a = u[:, hh, :DK, :] / u[:, hh, DK:DK + 1, :]   # [B, DK, S]
            UA[h * DK:(h + 1) * DK] = np.transpose(a, (1, 0, 2)).reshape(DK, B * S)

    nc3 = _get("outproj", _build_outproj)
    UAb = UA.astype(NPBF)
    woT = np.ascontiguousarray(np.asarray(Wo, np.float32).T.astype(NPBF))
    in3 = [
        {"at": np.ascontiguousarray(UAb[:, c * RPC:(c + 1) * RPC]), "wo": woT}
        for c in cores
    ]
    r3 = run_bass_kernel_spmd(nc3, in3, core_ids=cores)
    y = np.concatenate([r3.results[c]["y"] for c in cores], axis=0)
    y = y + np.asarray(bo, np.float32)[None, :]
    return y.reshape(B, S, D)


# revision 13
# speedup vs baseline: 1.0458x; 1.0009x over previous
"""Multi-head attention (B=4, S=2048, D=1024, H=16, causal) on 8 Trainium2
NeuronCores via Bass/Tile.

Three SPMD launches:
  L1  QKV projections, row-sharded: core c computes (x @ W.T + b)^T for its
      1/8 of the B*S rows, all three projections, output in [outcol, rows]
      (transposed) layout, bf16.
  L2  Attention, head-sharded: core c handles heads {2c, 2c+1} for all
      batches.  Scores are computed transposed (ST = K @ Q^T, [k, q] layout)
      so the softmax sum runs over PSUM partitions via a ones-column appended
      to V in the AV matmul - no on-chip transposes anywhere.  Causal
      structure skips upper-triangular score blocks; the triangular boundary
      is applied post-exp as a cheap 0/1 multiply on the [128,128] boundary
      strip of E plus memsets of fully-masked regions.
  L3  Output projection, row-sharded over the B*S rows.

Matmul operands are bf16 (1 cycle/row on the PE, half the DMA);
accumulation is fp32 in PSUM and the softmax denominators stay fp32.
Host work between launches is limited to reshaping/transposing shards and
the final denominator division (softmax normalization commutes with Wo).
"""

import sys

sys.path.insert(0, "/opt/trn_rl_repo")

import ml_dtypes
import numpy as np

import concourse.bacc as bacc
import concourse.tile as tile
from concourse import mybir
from concourse.bass_utils import run_bass_kernel_spmd

F32 = mybir.dt.float32
BF16 = mybir.dt.bfloat16
NPBF = ml_dtypes.bfloat16
EXP = mybir.ActivationFunctionType.Exp

B, S, D, H, DK = 4, 2048, 1024, 16, 64
NCORES = 8
HPC = H // NCORES          # heads per core (2)
RPC = B * S // NCORES      # rows per core in row-sharded launches (1024)
SCALE = 1.0 / np.sqrt(DK)  # folded into the exp activation
NEG = -1e30

_CACHE = {}


def _build_proj():
    """L1: yT = (x @ W.T + b)^T for q/k/v, row shard of 1024 rows."""
    nc = bacc.Bacc(trn_type="TRN2", target_bir_lowering=False)
    ins, outs = {}, {}
    for p in ("q", "k", "v"):
        ins[p] = (
            nc.dram_tensor(f"x{p}", [D, RPC], BF16, kind="ExternalInput"),
            nc.dram_tensor(f"w{p}", [D, D], BF16, kind="ExternalInput"),
            nc.dram_tensor(f"b{p}", [128, D // 128], F32, kind="ExternalInput"),
        )
        outs[p] = nc.dram_tensor(f"{p}t", [D, RPC], BF16, kind="ExternalOutput")

    KT, OCT, RC = D // 128, D // 128, RPC // 512  # 8 k-tiles, 8 oc-tiles, 2 chunks
    with tile.TileContext(nc) as tc:
        with (
            tc.tile_pool(name="big", bufs=2) as big,
            tc.tile_pool(name="bias", bufs=2) as bias,
            tc.tile_pool(name="outp", bufs=3) as outp,
            tc.tile_pool(name="ps", bufs=2, space="PSUM") as psp,
        ):
            wz = bias.tile([128, 512], BF16, tag="wz")
            nc.vector.memset(wz[:], 0.0)
            wp = psp.tile([128, RPC], F32, tag="ps", name="warm")
            for r in range(16):
                nc.tensor.matmul(wp[:, 0:512], wz[:, 0:128], wz[:, 0:512],
                                 start=(r == 0), stop=(r == 15))
            for p in ("q", "k", "v"):
                x_d, w_d, b_d = ins[p]
                xt = big.tile([128, KT, RPC], BF16, tag="xt")
                wt = big.tile([128, KT, D], BF16, tag="wt")
                bt = bias.tile([128, OCT], F32, tag="bt")
                for kt in range(KT):
                    nc.sync.dma_start(out=xt[:, kt, :],
                                      in_=x_d[kt * 128:(kt + 1) * 128, :])
                    nc.sync.dma_start(out=wt[:, kt, :],
                                      in_=w_d[kt * 128:(kt + 1) * 128, :])
                nc.sync.dma_start(out=bt[:], in_=b_d[:])
                for oc in range(OCT):
                    ps = psp.tile([128, RPC], F32, tag="ps")
                    for kt in range(KT):
                        lhs = wt[:, kt, oc * 128:(oc + 1) * 128]
                        for rc in range(RC):
                            nc.tensor.matmul(
                                ps[:, rc * 512:(rc + 1) * 512],
                                lhs,
                                xt[:, kt, rc * 512:(rc + 1) * 512],
                                start=(kt == 0),
                                stop=(kt == KT - 1),
                            )
                    ob = outp.tile([128, RPC], BF16, tag="ob")
                    nc.vector.tensor_scalar_add(ob[:], ps[:], bt[:, oc:oc + 1])
                    nc.sync.dma_start(
                        out=outs[p][oc * 128:(oc + 1) * 128, :], in_=ob[:]
                    )
    nc.compile()
    return nc


def _build_attn(causal):
    """L2: attention for 2 heads x 4 batches.

    qt/kt: [128, B*S] bf16 - head pair stacked on partitions (h0: 0-63,
    h1: 64-127), columns b*S+s.
    vp:    [B, 128, HPC, S//128, DK+1] bf16 - V with a ones column appended
           (vp[b, p, hh, t, c] = V'[b, head hh, k = t*128+p, c]).
    m01:   [128, 128] bf16 - causal 0/1 boundary strip (causal mode);
    maskb: [S//128, 128, S] f32 - additive bias in [k, q] layout (general).
    u:     [B, HPC, DK+1, S] f32 - rows 0-63 unnormalized A^T, row 64 the
           softmax denominator.
    """
    nc = bacc.Bacc(trn_type="TRN2", target_bir_lowering=False)
    qt_d = nc.dram_tensor("qt", [128, B * S], BF16, kind="ExternalInput")
    kt_d = nc.dram_tensor("kt", [128, B * S], BF16, kind="ExternalInput")
    vp_d = nc.dram_tensor("vp", [B, 128, HPC, S // 128, DK + 1], BF16,
                          kind="ExternalInput")
    if causal:
        mk_d = nc.dram_tensor("m01", [128, 128], BF16, kind="ExternalInput")
    else:
        mk_d = nc.dram_tensor("maskb", [S // 128, 128, S], F32,
                              kind="ExternalInput")
    u_d = nc.dram_tensor("u", [B, HPC, DK + 1, S], F32, kind="ExternalOutput")

    NJ = S // 512            # 4 q-chunks per batch
    NT = S // 128            # 16 k-tiles per batch
    with tile.TileContext(nc) as tc:
        with (
            tc.tile_pool(name="qk", bufs=1) as qk,
            tc.tile_pool(name="vpool", bufs=2) as vpool,
            tc.tile_pool(name="epool", bufs=6) as epool,
            tc.tile_pool(name="upool", bufs=3) as upool,
            tc.tile_pool(name="mpool", bufs=2) as mpool,
            tc.tile_pool(name="stp", bufs=3, space="PSUM") as stp,
            tc.tile_pool(name="otp", bufs=1, space="PSUM") as otp,
        ):
            qt = qk.tile([128, B * S], BF16, tag="qt")
            kt = qk.tile([128, B * S], BF16, tag="kt")
            nc.sync.dma_start(out=kt[:, 0:S], in_=kt_d[:, 0:S])
            nc.sync.dma_start(out=qt[:, 0:S], in_=qt_d[:, 0:S])
            vp_cur = vpool.tile([128, HPC, S // 128, DK + 1], BF16, tag="vp",
                                name="vp0")
            nc.sync.dma_start(out=vp_cur[:], in_=vp_d[0])
            for bb in range(1, B):
                bsl = slice(bb * S, (bb + 1) * S)
                nc.sync.dma_start(out=kt[:, bsl], in_=kt_d[:, bsl])
                nc.sync.dma_start(out=qt[:, bsl], in_=qt_d[:, bsl])
            if causal:
                mk = qk.tile([128, 128], BF16, tag="mk")
                nc.sync.dma_start(out=mk[:], in_=mk_d[:])
            wz = mpool.tile([128, 512], BF16, tag="wz")
            nc.vector.memset(wz[:], 0.0)
            wp = stp.tile([128, 1024], F32, tag="st", name="warm")
            for r in range(16):
                nc.tensor.matmul(wp[:, 0:512], wz[:, 0:128], wz[:, 0:512],
                                 start=(r == 0), stop=(r == 15))
            we = epool.tile([128, 1024], BF16, tag="e", name="warme")
            nc.scalar.activation(we[0:1, 0:8], wp[0:1, 0:8], EXP, scale=1.0)
            # Flat software-pipelined emission: the score matmuls + exp for
            # work item k+1 are emitted before the AV matmuls of item k, so
            # the PE never sits behind an exp-wait at chunk boundaries.
            items = []
            for b in range(B):
                for j in range(NJ):
                    ktiles = range(4 * j + 4) if causal else range(NT)
                    last_i = (4 * j + 3) if causal else (NT - 1)
                    for i in ktiles:
                        items.append((b, j, i, last_i))

            vp_tiles = {0: vp_cur}
            ots_map = {}

            def emit_av(b, j, i, last_i, e):
                if (b, j) not in ots_map:
                    ots_map[(b, j)] = [
                        otp.tile([DK + 1, 512], F32, tag=f"ot{hh}",
                                 name=f"ot{hh}_{b}_{j}") for hh in range(HPC)]
                ots = ots_map[(b, j)]
                for hh in range(HPC):
                    nc.tensor.matmul(
                        ots[hh][:],
                        vp_tiles[b][:, hh, i, :],
                        e[:, hh * 512:(hh + 1) * 512],
                        start=(i == 0),
                        stop=(i == last_i),
                    )
                if i == last_i:
                    for hh in range(HPC):
                        uc = upool.tile([DK + 1, 512], F32, tag=f"us{hh}",
                                        name=f"us{hh}_{b}_{j}")
                        nc.vector.tensor_copy(uc[:], ots[hh][:])
                        nc.sync.dma_start(
                            out=u_d[b, hh, :, j * 512:(j + 1) * 512], in_=uc[:])
                    del ots_map[(b, j)]

            pend = None
            for b, j, i, last_i in items:
                if b + 1 < B and b + 1 not in vp_tiles and (j, i) == (0, 0):
                    nv = vpool.tile([128, HPC, S // 128, DK + 1], BF16,
                                    tag="vp", name=f"vp{b + 1}")
                    nc.sync.dma_start(out=nv[:], in_=vp_d[b + 1])
                    vp_tiles[b + 1] = nv
                qsl = slice(b * S + j * 512, b * S + (j + 1) * 512)
                ksl = slice(b * S + i * 128, b * S + (i + 1) * 128)
                st = stp.tile([128, 1024], F32, tag="st")
                nc.tensor.matmul(st[:, 0:512], kt[0:64, ksl],
                                 qt[0:64, qsl], start=True, stop=True)
                nc.tensor.matmul(st[:, 512:1024], kt[64:128, ksl],
                                 qt[64:128, qsl], start=True, stop=True)
                if not causal:
                    mb = mpool.tile([128, 512], F32, tag="mb")
                    nc.sync.dma_start(
                        out=mb[:], in_=mk_d[i, :, j * 512:(j + 1) * 512])
                    nc.vector.tensor_add(st[:, 0:512], st[:, 0:512], mb[:])
                    nc.vector.tensor_add(st[:, 512:1024], st[:, 512:1024], mb[:])
                e = epool.tile([128, 1024], BF16, tag="e")
                diag = causal and i >= 4 * j
                off = (128 * i - 512 * j) if diag else 0
                if off >= 256:
                    # skip exp over the fully-masked leading columns
                    for hh in range(HPC):
                        o = hh * 512 + off
                        nc.scalar.activation(e[:, o:hh * 512 + 512],
                                             st[:, o:hh * 512 + 512],
                                             EXP, scale=float(SCALE))
                else:
                    nc.scalar.activation(e[:], st[:], EXP, scale=float(SCALE))
                if diag:
                    for hh in range(HPC):
                        o = hh * 512 + off
                        nc.vector.tensor_mul(
                            e[:, o:o + 128], e[:, o:o + 128], mk[:])
                        if off:
                            nc.vector.memset(e[:, hh * 512:hh * 512 + off], 0.0)
                if pend is not None:
                    emit_av(*pend)
                pend = (b, j, i, last_i, e)
            if pend is not None:
                emit_av(*pend)
    nc.compile()
    return nc


def _build_outproj():
    """L3: y = A @ Wo.T for a 1024-row shard (bias added on host)."""
    nc = bacc.Bacc(trn_type="TRN2", target_bir_lowering=False)
    at_d = nc.dram_tensor("at", [D, RPC], BF16, kind="ExternalInput")
    wo_d = nc.dram_tensor("wo", [D, D], BF16, kind="ExternalInput")
    y_d = nc.dram_tensor("y", [RPC, D], F32, kind="ExternalOutput")

    KT, RB = D // 128, RPC // 128
    with tile.TileContext(nc) as tc:
        with (
            tc.tile_pool(name="big", bufs=1) as big,
            tc.tile_pool(name="outp", bufs=3) as outp,
            tc.tile_pool(name="ps", bufs=2, space="PSUM") as psp,
        ):
            at = big.tile([128, KT, RPC], BF16, tag="at")
            wo = big.tile([128, KT, D], BF16, tag="wo")
            for kt in range(KT):
                nc.sync.dma_start(out=at[:, kt, :],
                                  in_=at_d[kt * 128:(kt + 1) * 128, :])
                nc.sync.dma_start(out=wo[:, kt, :],
                                  in_=wo_d[kt * 128:(kt + 1) * 128, :])
            wz = outp.tile([128, 512], BF16, tag="wz")
            nc.vector.memset(wz[:], 0.0)
            wp = psp.tile([128, D], F32, tag="ps", name="warm")
            for r in range(16):
                nc.tensor.matmul(wp[:, 0:512], wz[:, 0:128], wz[:, 0:512],
                                 start=(r == 0), stop=(r == 15))
            for rb in range(RB):
                ps = psp.tile([128, D], F32, tag="ps")
                for kt in range(KT):
                    lhs = at[:, kt, rb * 128:(rb + 1) * 128]
                    for oc in range(D // 512):
                        nc.tensor.matmul(
                            ps[:, oc * 512:(oc + 1) * 512],
                            lhs,
                            wo[:, kt, oc * 512:(oc + 1) * 512],
                            start=(kt == 0),
                            stop=(kt == KT - 1),
                        )
                ob = outp.tile([128, D], F32, tag="ob")
                nc.vector.tensor_copy(ob[:], ps[:])
                nc.sync.dma_start(out=y_d[rb * 128:(rb + 1) * 128, :], in_=ob[:])
    nc.compile()
    return nc


def _get(name, builder, *args):
    if name not in _CACHE:
        _CACHE[name] = builder(*args)
    return _CACHE[name]


def _strip_mask01():
    # m01[p, g] = 1 where the element (k = p, q = g) of the boundary strip is
    # causally valid (g >= p), else 0.
    p = np.arange(128)[:, None]
    g = np.arange(128)[None, :]
    return (g >= p).astype(NPBF)


def kernel(q, k, v, mask, Wq, bq, Wk, bk, Wv, bv, Wo, bo):
    q = np.asarray(q, dtype=np.float32)
    k = np.asarray(k, dtype=np.float32)
    v = np.asarray(v, dtype=np.float32)
    mask = np.asarray(mask)
    cores = list(range(NCORES))

    # ---------------- L1: QKV projections (row-sharded) ----------------
    nc1 = _get("proj", _build_proj)
    xqT = np.ascontiguousarray(q.reshape(B * S, D).T.astype(NPBF))   # [D, B*S]
    xkT = np.ascontiguousarray(k.reshape(B * S, D).T.astype(NPBF))
    xvT = np.ascontiguousarray(v.reshape(B * S, D).T.astype(NPBF))
    wqT = np.ascontiguousarray(np.asarray(Wq, np.float32).T.astype(NPBF))
    wkT = np.ascontiguousarray(np.asarray(Wk, np.float32).T.astype(NPBF))
    wvT = np.ascontiguousarray(np.asarray(Wv, np.float32).T.astype(NPBF))
    bqt = np.ascontiguousarray(np.asarray(bq, np.float32).reshape(D // 128, 128).T)
    bkt = np.ascontiguousarray(np.asarray(bk, np.float32).reshape(D // 128, 128).T)
    bvt = np.ascontiguousarray(np.asarray(bv, np.float32).reshape(D // 128, 128).T)
    in1 = [
        {
            "xq": np.ascontiguousarray(xqT[:, c * RPC:(c + 1) * RPC]),
            "xk": np.ascontiguousarray(xkT[:, c * RPC:(c + 1) * RPC]),
            "xv": np.ascontiguousarray(xvT[:, c * RPC:(c + 1) * RPC]),
            "wq": wqT, "wk": wkT, "wv": wvT,
            "bq": bqt, "bk": bkt, "bv": bvt,
        }
        for c in cores
    ]
    r1 = run_bass_kernel_spmd(nc1, in1, core_ids=cores)
    QT = np.concatenate([r1.results[c]["qt"] for c in cores], axis=1)  # [D, B*S]
    KTm = np.concatenate([r1.results[c]["kt"] for c in cores], axis=1)
    VT = np.concatenate([r1.results[c]["vt"] for c in cores], axis=1)

    # ---------------- L2: attention (head-sharded) ----------------------
    m2 = mask.reshape(S, S)
    causal = bool(np.array_equal(m2 != 0, np.tril(np.ones((S, S), bool))))
    allones = bool((m2 != 0).all())
    use_causal = causal and not allones
    nc2 = _get(("attn", use_causal), _build_attn, use_causal)

    # V' per core: [B, 128, HPC, S//128, DK+1]
    Vh = VT.reshape(H, DK, B, S)                       # [h, d, b, s]
    in2 = []
    for c in cores:
        vp = np.empty((B, 128, HPC, S // 128, DK + 1), NPBF)
        for hh in range(HPC):
            h = HPC * c + hh
            # [d, b, s] -> [b, s, d] -> [b, t, p, d]
            vb = np.transpose(Vh[h], (1, 2, 0)).reshape(B, S // 128, 128, DK)
            vp[:, :, hh, :, :DK] = np.transpose(vb, (0, 2, 1, 3))
            vp[:, :, hh, :, DK] = 1.0
        m = {
            "qt": np.ascontiguousarray(QT[c * 128:(c + 1) * 128]),
            "kt": np.ascontiguousarray(KTm[c * 128:(c + 1) * 128]),
            "vp": vp,
        }
        if use_causal:
            m["m01"] = _strip_mask01()
        else:
            bias = np.where(m2 != 0, 0.0, NEG).astype(np.float32)
            if allones:
                bias[:] = 0.0
            # biasT[k, q] layout, tiled [S//128, 128, S]
            m["maskb"] = np.ascontiguousarray(bias.T.reshape(S // 128, 128, S))
        in2.append(m)
    r2 = run_bass_kernel_spmd(nc2, in2, core_ids=cores)

    # ---------------- normalize + L3: output projection -----------------
    UA = np.empty((D, B * S), np.float32)  # A^T, normalized
    for c in cores:
        u = r2.results[c]["u"]             # [B, HPC, DK+1, S]
        for hh in range(HPC):
            h = HPC * c + hh
            a = u[:, hh, :DK, :] / u[:, hh, DK:DK + 1, :]   # [B, DK, S]
            UA[h * DK:(h + 1) * DK] = np.transpose(a, (1, 0, 2)).reshape(DK, B * S)

    nc3 = _get("outproj", _build_outproj)
    UAb = UA.astype(NPBF)
    woT = np.ascontiguousarray(np.asarray(Wo, np.float32).T.astype(NPBF))
    in3 = [
        {"at": np.ascontiguousarray(UAb[:, c * RPC:(c + 1) * RPC]), "wo": woT}
        for c in cores
    ]
    r3 = run_bass_kernel_spmd(nc3, in3, core_ids=cores)
    y = np.concatenate([r3.results[c]["y"] for c in cores], axis=0)
    y = y + np.asarray(bo, np.float32)[None, :]
    return y.reshape(B, S, D)


# revision 14
# speedup vs baseline: 1.0589x; 1.0126x over previous
"""Multi-head attention (B=4, S=2048, D=1024, H=16, causal) on 8 Trainium2
NeuronCores via Bass/Tile.

Three SPMD launches:
  L1  QKV projections, row-sharded: core c computes (x @ W.T + b)^T for its
      1/8 of the B*S rows, all three projections, output in [outcol, rows]
      (transposed) layout, bf16.
  L2  Attention, head-sharded: core c handles heads {2c, 2c+1} for all
      batches.  Scores are computed transposed (ST = K @ Q^T, [k, q] layout)
      so the softmax sum runs over PSUM partitions via a ones-column appended
      to V in the AV matmul - no on-chip transposes anywhere.  Causal
      structure skips upper-triangular score blocks; the triangular boundary
      is applied post-exp as a cheap 0/1 multiply on the [128,128] boundary
      strip of E plus memsets of fully-masked regions.
  L3  Output projection, row-sharded over the B*S rows.

Matmul operands are bf16 (1 cycle/row on the PE, half the DMA);
accumulation is fp32 in PSUM and the softmax denominators stay fp32.
Host work between launches is limited to reshaping/transposing shards and
the final denominator division (softmax normalization commutes with Wo).
"""

import sys

sys.path.insert(0, "/opt/trn_rl_repo")

import ml_dtypes
import numpy as np

import concourse.bacc as bacc
import concourse.tile as tile
from concourse import mybir
from concourse.bass_utils import run_bass_kernel_spmd

F32 = mybir.dt.float32
BF16 = mybir.dt.bfloat16
NPBF = ml_dtypes.bfloat16
EXP = mybir.ActivationFunctionType.Exp

B, S, D, H, DK = 4, 2048, 1024, 16, 64
NCORES = 8
HPC = H // NCORES          # heads per core (2)
RPC = B * S // NCORES      # rows per core in row-sharded launches (1024)
SCALE = 1.0 / np.sqrt(DK)  # folded into the exp activation
NEG = -1e30

_CACHE = {}


def _build_proj():
    """L1: yT = (x @ W.T + b)^T for q/k/v, row shard of 1024 rows."""
    nc = bacc.Bacc(trn_type="TRN2", target_bir_lowering=False)
    ins, outs = {}, {}
    for p in ("q", "k", "v"):
        ins[p] = (
            nc.dram_tensor(f"x{p}", [D, RPC], BF16, kind="ExternalInput"),
            nc.dram_tensor(f"w{p}", [D, D], BF16, kind="ExternalInput"),
            nc.dram_tensor(f"b{p}", [128, D // 128], F32, kind="ExternalInput"),
        )
        outs[p] = nc.dram_tensor(f"{p}t", [D, RPC], BF16, kind="ExternalOutput")

    KT, OCT, RC = D // 128, D // 128, RPC // 512  # 8 k-tiles, 8 oc-tiles, 2 chunks
    with tile.TileContext(nc) as tc:
        with (
            tc.tile_pool(name="big", bufs=2) as big,
            tc.tile_pool(name="bias", bufs=2) as bias,
            tc.tile_pool(name="outp", bufs=3) as outp,
            tc.tile_pool(name="ps", bufs=2, space="PSUM") as psp,
        ):
            wz = bias.tile([128, 512], BF16, tag="wz")
            nc.vector.memset(wz[:], 0.0)
            wp = psp.tile([128, RPC], F32, tag="ps", name="warm")
            for r in range(16):
                nc.tensor.matmul(wp[:, 0:512], wz[:, 0:128], wz[:, 0:512],
                                 start=(r == 0), stop=(r == 15))
            for p in ("q", "k", "v"):
                x_d, w_d, b_d = ins[p]
                xt = big.tile([128, KT, RPC], BF16, tag="xt")
                wt = big.tile([128, KT, D], BF16, tag="wt")
                bt = bias.tile([128, OCT], F32, tag="bt")
                for kt in range(KT):
                    nc.sync.dma_start(out=xt[:, kt, :],
                                      in_=x_d[kt * 128:(kt + 1) * 128, :])
                    nc.sync.dma_start(out=wt[:, kt, :],
                                      in_=w_d[kt * 128:(kt + 1) * 128, :])
                nc.sync.dma_start(out=bt[:], in_=b_d[:])
                for oc in range(OCT):
                    ps = psp.tile([128, RPC], F32, tag="ps")
                    for kt in range(KT):
                        lhs = wt[:, kt, oc * 128:(oc + 1) * 128]
                        for rc in range(RC):
                            nc.tensor.matmul(
                                ps[:, rc * 512:(rc + 1) * 512],
                                lhs,
                                xt[:, kt, rc * 512:(rc + 1) * 512],
                                start=(kt == 0),
                                stop=(kt == KT - 1),
                            )
                    ob = outp.tile([128, RPC], BF16, tag="ob")
                    nc.vector.tensor_scalar_add(ob[:], ps[:], bt[:, oc:oc + 1])
                    nc.sync.dma_start(
                        out=outs[p][oc * 128:(oc + 1) * 128, :], in_=ob[:]
                    )
    nc.compile()
    return nc


def _build_attn(causal):
    """L2: attention for 2 heads x 4 batches.

    qt/kt: [128, B*S] bf16 - head pair stacked on partitions (h0: 0-63,
    h1: 64-127), columns b*S+s.
    vp:    [B, 128, HPC, S//128, DK+1] bf16 - V with a ones column appended
           (vp[b, p, hh, t, c] = V'[b, head hh, k = t*128+p, c]).
    m01:   [128, 128] bf16 - causal 0/1 boundary strip (causal mode);
    maskb: [S//128, 128, S] f32 - additive bias in [k, q] layout (general).
    u:     [B, HPC, DK+1, S] f32 - rows 0-63 unnormalized A^T, row 64 the
           softmax denominator.
    """
    nc = bacc.Bacc(trn_type="TRN2", target_bir_lowering=False)
    qt_d = nc.dram_tensor("qt", [128, B * S], BF16, kind="ExternalInput")
    kt_d = nc.dram_tensor("kt", [128, B * S], BF16, kind="ExternalInput")
    vp_d = nc.dram_tensor("vp", [B, 128, HPC, S // 128, DK + 1], BF16,
                          kind="ExternalInput")
    if causal:
        mk_d = nc.dram_tensor("m01", [128, 128], BF16, kind="ExternalInput")
    else:
        mk_d = nc.dram_tensor("maskb", [S // 128, 128, S], F32,
                              kind="ExternalInput")
    u_d = nc.dram_tensor("u", [B, HPC, DK + 1, S], F32, kind="ExternalOutput")

    NJ = S // 512            # 4 q-chunks per batch
    NT = S // 128            # 16 k-tiles per batch
    with tile.TileContext(nc) as tc:
        with (
            tc.tile_pool(name="qk", bufs=1) as qk,
            tc.tile_pool(name="vpool", bufs=2) as vpool,
            tc.tile_pool(name="epool", bufs=8) as epool,
            tc.tile_pool(name="upool", bufs=3) as upool,
            tc.tile_pool(name="mpool", bufs=2) as mpool,
            tc.tile_pool(name="stp", bufs=3, space="PSUM") as stp,
            tc.tile_pool(name="otp", bufs=1, space="PSUM") as otp,
        ):
            qt = qk.tile([128, B * S], BF16, tag="qt")
            kt = qk.tile([128, B * S], BF16, tag="kt")
            nc.sync.dma_start(out=kt[:, 0:512], in_=kt_d[:, 0:512])
            nc.sync.dma_start(out=qt[:, 0:512], in_=qt_d[:, 0:512])
            nc.sync.dma_start(out=kt[:, 512:S], in_=kt_d[:, 512:S])
            nc.sync.dma_start(out=qt[:, 512:S], in_=qt_d[:, 512:S])
            vp_cur = vpool.tile([128, HPC, S // 128, DK + 1], BF16, tag="vp",
                                name="vp0")
            nc.sync.dma_start(out=vp_cur[:], in_=vp_d[0])
            for bb in range(1, B):
                bsl = slice(bb * S, (bb + 1) * S)
                nc.sync.dma_start(out=kt[:, bsl], in_=kt_d[:, bsl])
                nc.sync.dma_start(out=qt[:, bsl], in_=qt_d[:, bsl])
            if causal:
                mk = qk.tile([128, 128], BF16, tag="mk")
                nc.sync.dma_start(out=mk[:], in_=mk_d[:])
            wz = mpool.tile([128, 512], BF16, tag="wz")
            nc.vector.memset(wz[:], 0.0)
            wp = stp.tile([128, 1024], F32, tag="st", name="warm")
            for r in range(16):
                nc.tensor.matmul(wp[:, 0:512], wz[:, 0:128], wz[:, 0:512],
                                 start=(r == 0), stop=(r == 15))
            we = epool.tile([128, 1024], BF16, tag="e", name="warme")
            nc.scalar.activation(we[0:1, 0:8], wp[0:1, 0:8], EXP, scale=1.0)
            # Flat software-pipelined emission: the score matmuls + exp for
            # work item k+1 are emitted before the AV matmuls of item k, so
            # the PE never sits behind an exp-wait at chunk boundaries.
            items = []
            for b in range(B):
                for j in range(NJ):
                    ktiles = range(4 * j + 4) if causal else range(NT)
                    last_i = (4 * j + 3) if causal else (NT - 1)
                    for i in ktiles:
                        items.append((b, j, i, last_i))

            vp_tiles = {0: vp_cur}
            ots_map = {}

            def emit_av(b, j, i, last_i, e):
                if (b, j) not in ots_map:
                    ots_map[(b, j)] = [
                        otp.tile([DK + 1, 512], F32, tag=f"ot{hh}",
                                 name=f"ot{hh}_{b}_{j}") for hh in range(HPC)]
                ots = ots_map[(b, j)]
                for hh in range(HPC):
                    nc.tensor.matmul(
                        ots[hh][:],
                        vp_tiles[b][:, hh, i, :],
                        e[:, hh * 512:(hh + 1) * 512],
                        start=(i == 0),
                        stop=(i == last_i),
                    )
                if i == last_i:
                    for hh in range(HPC):
                        uc = upool.tile([DK + 1, 512], F32, tag=f"us{hh}",
                                        name=f"us{hh}_{b}_{j}")
                        nc.vector.tensor_copy(uc[:], ots[hh][:])
                        nc.sync.dma_start(
                            out=u_d[b, hh, :, j * 512:(j + 1) * 512], in_=uc[:])
                    del ots_map[(b, j)]

            pend = None
            for b, j, i, last_i in items:
                if b + 1 < B and b + 1 not in vp_tiles and (j, i) == (0, 0):
                    nv = vpool.tile([128, HPC, S // 128, DK + 1], BF16,
                                    tag="vp", name=f"vp{b + 1}")
                    nc.sync.dma_start(out=nv[:], in_=vp_d[b + 1])
                    vp_tiles[b + 1] = nv
                qsl = slice(b * S + j * 512, b * S + (j + 1) * 512)
                ksl = slice(b * S + i * 128, b * S + (i + 1) * 128)
                st = stp.tile([128, 1024], F32, tag="st")
                nc.tensor.matmul(st[:, 0:512], kt[0:64, ksl],
                                 qt[0:64, qsl], start=True, stop=True)
                nc.tensor.matmul(st[:, 512:1024], kt[64:128, ksl],
                                 qt[64:128, qsl], start=True, stop=True)
                if not causal:
                    mb = mpool.tile([128, 512], F32, tag="mb")
                    nc.sync.dma_start(
                        out=mb[:], in_=mk_d[i, :, j * 512:(j + 1) * 512])
                    nc.vector.tensor_add(st[:, 0:512], st[:, 0:512], mb[:])
                    nc.vector.tensor_add(st[:, 512:1024], st[:, 512:1024], mb[:])
                e = epool.tile([128, 1024], BF16, tag="e")
                diag = causal and i >= 4 * j
                off = (128 * i - 512 * j) if diag else 0
                if off >= 256:
                    # skip exp over the fully-masked leading columns
                    for hh in range(HPC):
                        o = hh * 512 + off
                        nc.scalar.activation(e[:, o:hh * 512 + 512],
                                             st[:, o:hh * 512 + 512],
                                             EXP, scale=float(SCALE))
                else:
                    nc.scalar.activation(e[:], st[:], EXP, scale=float(SCALE))
                if diag:
                    for hh in range(HPC):
                        o = hh * 512 + off
                        nc.vector.tensor_mul(
                            e[:, o:o + 128], e[:, o:o + 128], mk[:])
                        if off:
                            nc.vector.memset(e[:, hh * 512:hh * 512 + off], 0.0)
                if pend is not None:
                    emit_av(*pend)
                pend = (b, j, i, last_i, e)
            if pend is not None:
                emit_av(*pend)
    nc.compile()
    return nc


def _build_outproj():
    """L3: y = A @ Wo.T for a 1024-row shard (bias added on host)."""
    nc = bacc.Bacc(trn_type="TRN2", target_bir_lowering=False)
    at_d = nc.dram_tensor("at", [D, RPC], BF16, kind="ExternalInput")
    wo_d = nc.dram_tensor("wo", [D, D], BF16, kind="ExternalInput")
    y_d = nc.dram_tensor("y", [RPC, D], F32, kind="ExternalOutput")

    KT, RB = D // 128, RPC // 128
    with tile.TileContext(nc) as tc:
        with (
            tc.tile_pool(name="big", bufs=1) as big,
            tc.tile_pool(name="outp", bufs=3) as outp,
            tc.tile_pool(name="ps", bufs=2, space="PSUM") as psp,
        ):
            at = big.tile([128, KT, RPC], BF16, tag="at")
            wo = big.tile([128, KT, D], BF16, tag="wo")
            for kt in range(KT):
                nc.sync.dma_start(out=at[:, kt, :],
                                  in_=at_d[kt * 128:(kt + 1) * 128, :])
                nc.sync.dma_start(out=wo[:, kt, :],
                                  in_=wo_d[kt * 128:(kt + 1) * 128, :])
            wz = outp.tile([128, 512], BF16, tag="wz")
            nc.vector.memset(wz[:], 0.0)
            wp = psp.tile([128, D], F32, tag="ps", name="warm")
            for r in range(16):
                nc.tensor.matmul(wp[:, 0:512], wz[:, 0:128], wz[:, 0:512],
                                 start=(r == 0), stop=(r == 15))
            for rb in range(RB):
                ps = psp.tile([128, D], F32, tag="ps")
                for kt in range(KT):
                    lhs = at[:, kt, rb * 128:(rb + 1) * 128]
                    for oc in range(D // 512):
                        nc.tensor.matmul(
                            ps[:, oc * 512:(oc + 1) * 512],
                            lhs,
                            wo[:, kt, oc * 512:(oc + 1) * 512],
                            start=(kt == 0),
                            stop=(kt == KT - 1),
                        )
                ob = outp.tile([128, D], F32, tag="ob")
                nc.vector.tensor_copy(ob[:], ps[:])
                nc.sync.dma_start(out=y_d[rb * 128:(rb + 1) * 128, :], in_=ob[:])
    nc.compile()
    return nc


def _get(name, builder, *args):
    if name not in _CACHE:
        _CACHE[name] = builder(*args)
    return _CACHE[name]


def _strip_mask01():
    # m01[p, g] = 1 where the element (k = p, q = g) of the boundary strip is
    # causally valid (g >= p), else 0.
    p = np.arange(128)[:, None]
    g = np.arange(128)[None, :]
    return (g >= p).astype(NPBF)


def kernel(q, k, v, mask, Wq, bq, Wk, bk, Wv, bv, Wo, bo):
    q = np.asarray(q, dtype=np.float32)
    k = np.asarray(k, dtype=np.float32)
    v = np.asarray(v, dtype=np.float32)
    mask = np.asarray(mask)
    cores = list(range(NCORES))

    # ---------------- L1: QKV projections (row-sharded) ----------------
    nc1 = _get("proj", _build_proj)
    xqT = np.ascontiguousarray(q.reshape(B * S, D).T.astype(NPBF))   # [D, B*S]
    xkT = np.ascontiguousarray(k.reshape(B * S, D).T.astype(NPBF))
    xvT = np.ascontiguousarray(v.reshape(B * S, D).T.astype(NPBF))
    wqT = np.ascontiguousarray(np.asarray(Wq, np.float32).T.astype(NPBF))
    wkT = np.ascontiguousarray(np.asarray(Wk, np.float32).T.astype(NPBF))
    wvT = np.ascontiguousarray(np.asarray(Wv, np.float32).T.astype(NPBF))
    bqt = np.ascontiguousarray(np.asarray(bq, np.float32).reshape(D // 128, 128).T)
    bkt = np.ascontiguousarray(np.asarray(bk, np.float32).reshape(D // 128, 128).T)
    bvt = np.ascontiguousarray(np.asarray(bv, np.float32).reshape(D // 128, 128).T)
    in1 = [
        {
            "xq": np.ascontiguousarray(xqT[:, c * RPC:(c + 1) * RPC]),
            "xk": np.ascontiguousarray(xkT[:, c * RPC:(c + 1) * RPC]),
            "xv": np.ascontiguousarray(xvT[:, c * RPC:(c + 1) * RPC]),
            "wq": wqT, "wk": wkT, "wv": wvT,
            "bq": bqt, "bk": bkt, "bv": bvt,
        }
        for c in cores
    ]
    r1 = run_bass_kernel_spmd(nc1, in1, core_ids=cores)
    QT = np.concatenate([r1.results[c]["qt"] for c in cores], axis=1)  # [D, B*S]
    KTm = np.concatenate([r1.results[c]["kt"] for c in cores], axis=1)
    VT = np.concatenate([r1.results[c]["vt"] for c in cores], axis=1)

    # ---------------- L2: attention (head-sharded) ----------------------
    m2 = mask.reshape(S, S)
    causal = bool(np.array_equal(m2 != 0, np.tril(np.ones((S, S), bool))))
    allones = bool((m2 != 0).all())
    use_causal = causal and not allones
    nc2 = _get(("attn", use_causal), _build_attn, use_causal)

    # V' per core: [B, 128, HPC, S//128, DK+1]
    Vh = VT.reshape(H, DK, B, S)                       # [h, d, b, s]
    in2 = []
    for c in cores:
        vp = np.empty((B, 128, HPC, S // 128, DK + 1), NPBF)
        for hh in range(HPC):
            h = HPC * c + hh
            # [d, b, s] -> [b, s, d] -> [b, t, p, d]
            vb = np.transpose(Vh[h], (1, 2, 0)).reshape(B, S // 128, 128, DK)
            vp[:, :, hh, :, :DK] = np.transpose(vb, (0, 2, 1, 3))
            vp[:, :, hh, :, DK] = 1.0
        m = {
            "qt": np.ascontiguousarray(QT[c * 128:(c + 1) * 128]),
            "kt": np.ascontiguousarray(KTm[c * 128:(c + 1) * 128]),
            "vp": vp,
        }
        if use_causal:
            m["m01"] = _strip_mask01()
        else:
            bias = np.where(m2 != 0, 0.0, NEG).astype(np.float32)
            if allones:
                bias[:] = 0.0
            # biasT[k, q] layout, tiled [S//128, 128, S]
            m["maskb"] = np.ascontiguousarray(bias.T.reshape(S // 128, 128, S))
        in2.append(m)
    r2 = run_bass_kernel_spmd(nc2, in2, core_ids=cores)

    # ---------------- normalize + L3: output projection -----------------
    UA = np.empty((D, B * S), np.float32)  # A^T, normalized
    for c in cores:
        u = r2.results[c]["u"]             # [B, HPC, DK+1, S]
        for hh in range(HPC):
            h = HPC * c + hh
            a = u[:, hh, :DK, :] / u[:, hh, DK:DK + 1, :]   # [B, DK, S]
            UA[h * DK:(h + 1) * DK] = np.transpose(a, (1, 0, 2)).reshape(DK, B * S)

    nc3 = _get("outproj", _build_outproj)
    UAb = UA.astype(NPBF)
    woT = np.ascontiguousarray(np.asarray(Wo, np.float32).T.astype(NPBF))
    in3 = [
        {"at": np.ascontiguousarray(UAb[:, c * RPC:(c + 1) * RPC]), "wo": woT}
        for c in cores
    ]
    r3 = run_bass_kernel_spmd(nc3, in3, core_ids=cores)
    y = np.concatenate([r3.results[c]["y"] for c in cores], axis=0)
    y = y + np.asarray(bo, np.float32)[None, :]
    return y.reshape(B, S, D)


# revision 15
# speedup vs baseline: 1.0610x; 1.0020x over previous
"""Multi-head attention (B=4, S=2048, D=1024, H=16, causal) on 8 Trainium2
NeuronCores via Bass/Tile.

Three SPMD launches:
  L1  QKV projections, row-sharded: core c computes (x @ W.T + b)^T for its
      1/8 of the B*S rows, all three projections, output in [outcol, rows]
      (transposed) layout, bf16.
  L2  Attention, head-sharded: core c handles heads {2c, 2c+1} for all
      batches.  Scores are computed transposed (ST = K @ Q^T, [k, q] layout)
      so the softmax sum runs over PSUM partitions via a ones-column appended
      to V in the AV matmul - no on-chip transposes anywhere.  Causal
      structure skips upper-triangular score blocks; the triangular boundary
      is applied post-exp as a cheap 0/1 multiply on the [128,128] boundary
      strip of E plus memsets of fully-masked regions.
  L3  Output projection, row-sharded over the B*S rows.

Matmul operands are bf16 (1 cycle/row on the PE, half the DMA);
accumulation is fp32 in PSUM and the softmax denominators stay fp32.
Host work between launches is limited to reshaping/transposing shards and
the final denominator division (softmax normalization commutes with Wo).
"""

import sys

sys.path.insert(0, "/opt/trn_rl_repo")

import ml_dtypes
import numpy as np

import concourse.bacc as bacc
import concourse.tile as tile
from concourse import mybir
from concourse.bass_utils import run_bass_kernel_spmd

F32 = mybir.dt.float32
BF16 = mybir.dt.bfloat16
NPBF = ml_dtypes.bfloat16
EXP = mybir.ActivationFunctionType.Exp

B, S, D, H, DK = 4, 2048, 1024, 16, 64
NCORES = 8
HPC = H // NCORES          # heads per core (2)
RPC = B * S // NCORES      # rows per core in row-sharded launches (1024)
SCALE = 1.0 / np.sqrt(DK)  # folded into the exp activation
NEG = -1e30

_CACHE = {}


def _build_proj():
    """L1: yT = (x @ W.T + b)^T for q/k/v, row shard of 1024 rows."""
    nc = bacc.Bacc(trn_type="TRN2", target_bir_lowering=False)
    ins, outs = {}, {}
    for p in ("q", "k", "v"):
        ins[p] = (
            nc.dram_tensor(f"x{p}", [D, RPC], BF16, kind="ExternalInput"),
            nc.dram_tensor(f"w{p}", [D, D], BF16, kind="ExternalInput"),
            nc.dram_tensor(f"b{p}", [128, D // 128], F32, kind="ExternalInput"),
        )
        outs[p] = nc.dram_tensor(f"{p}t", [D, RPC], BF16, kind="ExternalOutput")

    KT, OCT, RC = D // 128, D // 128, RPC // 512  # 8 k-tiles, 8 oc-tiles, 2 chunks
    with tile.TileContext(nc) as tc:
        with (
            tc.tile_pool(name="big", bufs=2) as big,
            tc.tile_pool(name="bias", bufs=2) as bias,
            tc.tile_pool(name="outp", bufs=3) as outp,
            tc.tile_pool(name="ps", bufs=2, space="PSUM") as psp,
        ):
            wz = bias.tile([128, 512], BF16, tag="wz")
            nc.vector.memset(wz[:], 0.0)
            wp = psp.tile([128, RPC], F32, tag="ps", name="warm")
            for r in range(16):
                nc.tensor.matmul(wp[:, 0:512], wz[:, 0:128], wz[:, 0:512],
                                 start=(r == 0), stop=(r == 15))
            for p in ("q", "k", "v"):
                x_d, w_d, b_d = ins[p]
                xt = big.tile([128, KT, RPC], BF16, tag="xt")
                wt = big.tile([128, KT, D], BF16, tag="wt")
                bt = bias.tile([128, OCT], F32, tag="bt")
                for kt in range(KT):
                    nc.sync.dma_start(out=xt[:, kt, :],
                                      in_=x_d[kt * 128:(kt + 1) * 128, :])
                    nc.sync.dma_start(out=wt[:, kt, :],
                                      in_=w_d[kt * 128:(kt + 1) * 128, :])
                nc.sync.dma_start(out=bt[:], in_=b_d[:])
                for oc in range(OCT):
                    ps = psp.tile([128, RPC], F32, tag="ps")
                    for kt in range(KT):
                        lhs = wt[:, kt, oc * 128:(oc + 1) * 128]
                        for rc in range(RC):
                            nc.tensor.matmul(
                                ps[:, rc * 512:(rc + 1) * 512],
                                lhs,
                                xt[:, kt, rc * 512:(rc + 1) * 512],
                                start=(kt == 0),
                                stop=(kt == KT - 1),
                            )
                    ob = outp.tile([128, RPC], BF16, tag="ob")
                    nc.vector.tensor_scalar_add(ob[:], ps[:], bt[:, oc:oc + 1])
                    nc.sync.dma_start(
                        out=outs[p][oc * 128:(oc + 1) * 128, :], in_=ob[:]
                    )
    nc.compile()
    return nc


def _build_attn(causal):
    """L2: attention for 2 heads x 4 batches.

    qt/kt: [128, B*S] bf16 - head pair stacked on partitions (h0: 0-63,
    h1: 64-127), columns b*S+s.
    vp:    [B, 128, HPC, S//128, DK+1] bf16 - V with a ones column appended
           (vp[b, p, hh, t, c] = V'[b, head hh, k = t*128+p, c]).
    m01:   [128, 128] bf16 - causal 0/1 boundary strip (causal mode);
    maskb: [S//128, 128, S] f32 - additive bias in [k, q] layout (general).
    u:     [B, HPC, DK+1, S] f32 - rows 0-63 unnormalized A^T, row 64 the
           softmax denominator.
    """
    nc = bacc.Bacc(trn_type="TRN2", target_bir_lowering=False)
    qt_d = nc.dram_tensor("qt", [128, B * S], BF16, kind="ExternalInput")
    kt_d = nc.dram_tensor("kt", [128, B * S], BF16, kind="ExternalInput")
    vp_d = nc.dram_tensor("vp", [B, 128, HPC, S // 128, DK + 1], BF16,
                          kind="ExternalInput")
    if causal:
        mk_d = nc.dram_tensor("m01", [128, 128], BF16, kind="ExternalInput")
    else:
        mk_d = nc.dram_tensor("maskb", [S // 128, 128, S], F32,
                              kind="ExternalInput")
    u_d = nc.dram_tensor("u", [B, HPC, DK + 1, S], F32, kind="ExternalOutput")

    NJ = S // 512            # 4 q-chunks per batch
    NT = S // 128            # 16 k-tiles per batch
    with tile.TileContext(nc) as tc:
        with (
            tc.tile_pool(name="qk", bufs=1) as qk,
            tc.tile_pool(name="vpool", bufs=2) as vpool,
            tc.tile_pool(name="epool", bufs=8) as epool,
            tc.tile_pool(name="upool", bufs=3) as upool,
            tc.tile_pool(name="mpool", bufs=2) as mpool,
            tc.tile_pool(name="stp", bufs=3, space="PSUM") as stp,
            tc.tile_pool(name="otp", bufs=1, space="PSUM") as otp,
        ):
            qt = qk.tile([128, B * S], BF16, tag="qt")
            kt = qk.tile([128, B * S], BF16, tag="kt")
            nc.sync.dma_start(out=kt[:, 0:512], in_=kt_d[:, 0:512])
            nc.sync.dma_start(out=qt[:, 0:512], in_=qt_d[:, 0:512])
            nc.sync.dma_start(out=kt[:, 512:S], in_=kt_d[:, 512:S])
            nc.sync.dma_start(out=qt[:, 512:S], in_=qt_d[:, 512:S])
            vp_cur = vpool.tile([128, HPC, S // 128, DK + 1], BF16, tag="vp",
                                name="vp0")
            nc.sync.dma_start(out=vp_cur[:], in_=vp_d[0])
            for bb in range(1, B):
                bsl = slice(bb * S, (bb + 1) * S)
                nc.sync.dma_start(out=kt[:, bsl], in_=kt_d[:, bsl])
                nc.sync.dma_start(out=qt[:, bsl], in_=qt_d[:, bsl])
            if causal:
                mk = qk.tile([128, 128], BF16, tag="mk")
                nc.sync.dma_start(out=mk[:], in_=mk_d[:])
            wz = mpool.tile([128, 512], BF16, tag="wz")
            nc.vector.memset(wz[:], 0.0)
            wp = stp.tile([128, 1024], F32, tag="st", name="warm")
            for r in range(16):
                nc.tensor.matmul(wp[:, 0:512], wz[:, 0:128], wz[:, 0:512],
                                 start=(r == 0), stop=(r == 15))
            we = epool.tile([128, 1024], BF16, tag="e", name="warme")
            nc.scalar.activation(we[0:1, 0:8], wp[0:1, 0:8], EXP, scale=1.0)
            # Flat software-pipelined emission: the score matmuls + exp for
            # work item k+1 are emitted before the AV matmuls of item k, so
            # the PE never sits behind an exp-wait at chunk boundaries.
            items = []
            for b in range(B):
                for j in range(NJ):
                    ktiles = range(4 * j + 4) if causal else range(NT)
                    last_i = (4 * j + 3) if causal else (NT - 1)
                    for i in ktiles:
                        items.append((b, j, i, last_i))

            vp_tiles = {0: vp_cur}
            ots_map = {}

            def emit_av(b, j, i, last_i, e):
                if (b, j) not in ots_map:
                    ots_map[(b, j)] = [
                        otp.tile([DK + 1, 512], F32, tag=f"ot{hh}",
                                 name=f"ot{hh}_{b}_{j}") for hh in range(HPC)]
                ots = ots_map[(b, j)]
                for hh in range(HPC):
                    nc.tensor.matmul(
                        ots[hh][:],
                        vp_tiles[b][:, hh, i, :],
                        e[:, hh * 512:(hh + 1) * 512],
                        start=(i == 0),
                        stop=(i == last_i),
                    )
                if i == last_i:
                    for hh in range(HPC):
                        uc = upool.tile([DK + 1, 512], F32, tag=f"us{hh}",
                                        name=f"us{hh}_{b}_{j}")
                        nc.vector.tensor_copy(uc[:], ots[hh][:])
                        nc.sync.dma_start(
                            out=u_d[b, hh, :, j * 512:(j + 1) * 512], in_=uc[:])
                    del ots_map[(b, j)]

            pend = None
            for b, j, i, last_i in items:
                if b + 1 < B and b + 1 not in vp_tiles and (j, i) == (0, 0):
                    nv = vpool.tile([128, HPC, S // 128, DK + 1], BF16,
                                    tag="vp", name=f"vp{b + 1}")
                    nc.sync.dma_start(out=nv[:], in_=vp_d[b + 1])
                    vp_tiles[b + 1] = nv
                qsl = slice(b * S + j * 512, b * S + (j + 1) * 512)
                ksl = slice(b * S + i * 128, b * S + (i + 1) * 128)
                st = stp.tile([128, 1024], F32, tag="st")
                nc.tensor.matmul(st[:, 0:512], kt[0:64, ksl],
                                 qt[0:64, qsl], start=True, stop=True)
                nc.tensor.matmul(st[:, 512:1024], kt[64:128, ksl],
                                 qt[64:128, qsl], start=True, stop=True)
                if not causal:
                    mb = mpool.tile([128, 512], F32, tag="mb")
                    nc.sync.dma_start(
                        out=mb[:], in_=mk_d[i, :, j * 512:(j + 1) * 512])
                    nc.vector.tensor_add(st[:, 0:512], st[:, 0:512], mb[:])
                    nc.vector.tensor_add(st[:, 512:1024], st[:, 512:1024], mb[:])
                e = epool.tile([128, 1024], BF16, tag="e")
                diag = causal and i >= 4 * j
                off = (128 * i - 512 * j) if diag else 0
                if off >= 256:
                    # skip exp over the fully-masked leading columns
                    for hh in range(HPC):
                        o = hh * 512 + off
                        nc.scalar.activation(e[:, o:hh * 512 + 512],
                                             st[:, o:hh * 512 + 512],
                                             EXP, scale=float(SCALE))
                else:
                    nc.scalar.activation(e[:], st[:], EXP, scale=float(SCALE))
                if pend is not None:
                    emit_av(*pend)
                if diag:
                    for hh in range(HPC):
                        o = hh * 512 + off
                        nc.vector.tensor_mul(
                            e[:, o:o + 128], e[:, o:o + 128], mk[:])
                        if off:
                            nc.vector.memset(e[:, hh * 512:hh * 512 + off], 0.0)
                pend = (b, j, i, last_i, e)
            if pend is not None:
                emit_av(*pend)
    nc.compile()
    return nc


def _build_outproj():
    """L3: y = A @ Wo.T for a 1024-row shard (bias added on host)."""
    nc = bacc.Bacc(trn_type="TRN2", target_bir_lowering=False)
    at_d = nc.dram_tensor("at", [D, RPC], BF16, kind="ExternalInput")
    wo_d = nc.dram_tensor("wo", [D, D], BF16, kind="ExternalInput")
    y_d = nc.dram_tensor("y", [RPC, D], F32, kind="ExternalOutput")

    KT, RB = D // 128, RPC // 128
    with tile.TileContext(nc) as tc:
        with (
            tc.tile_pool(name="big", bufs=1) as big,
            tc.tile_pool(name="outp", bufs=3) as outp,
            tc.tile_pool(name="ps", bufs=2, space="PSUM") as psp,
        ):
            at = big.tile([128, KT, RPC], BF16, tag="at")
            wo = big.tile([128, KT, D], BF16, tag="wo")
            for kt in range(KT):
                nc.sync.dma_start(out=at[:, kt, :],
                                  in_=at_d[kt * 128:(kt + 1) * 128, :])
                nc.sync.dma_start(out=wo[:, kt, :],
                                  in_=wo_d[kt * 128:(kt + 1) * 128, :])
            wz = outp.tile([128, 512], BF16, tag="wz")
            nc.vector.memset(wz[:], 0.0)
            wp = psp.tile([128, D], F32, tag="ps", name="warm")
            for r in range(16):
                nc.tensor.matmul(wp[:, 0:512], wz[:, 0:128], wz[:, 0:512],
                                 start=(r == 0), stop=(r == 15))
            for rb in range(RB):
                ps = psp.tile([128, D], F32, tag="ps")
                for kt in range(KT):
                    lhs = at[:, kt, rb * 128:(rb + 1) * 128]
                    for oc in range(D // 512):
                        nc.tensor.matmul(
                            ps[:, oc * 512:(oc + 1) * 512],
                            lhs,
                            wo[:, kt, oc * 512:(oc + 1) * 512],
                            start=(kt == 0),
                            stop=(kt == KT - 1),
                        )
                ob = outp.tile([128, D], F32, tag="ob")
                nc.vector.tensor_copy(ob[:], ps[:])
                nc.sync.dma_start(out=y_d[rb * 128:(rb + 1) * 128, :], in_=ob[:])
    nc.compile()
    return nc


def _get(name, builder, *args):
    if name not in _CACHE:
        _CACHE[name] = builder(*args)
    return _CACHE[name]


def _strip_mask01():
    # m01[p, g] = 1 where the element (k = p, q = g) of the boundary strip is
    # causally valid (g >= p), else 0.
    p = np.arange(128)[:, None]
    g = np.arange(128)[None, :]
    return (g >= p).astype(NPBF)


def kernel(q, k, v, mask, Wq, bq, Wk, bk, Wv, bv, Wo, bo):
    q = np.asarray(q, dtype=np.float32)
    k = np.asarray(k, dtype=np.float32)
    v = np.asarray(v, dtype=np.float32)
    mask = np.asarray(mask)
    cores = list(range(NCORES))

    # ---------------- L1: QKV projections (row-sharded) ----------------
    nc1 = _get("proj", _build_proj)
    xqT = np.ascontiguousarray(q.reshape(B * S, D).T.astype(NPBF))   # [D, B*S]
    xkT = np.ascontiguousarray(k.reshape(B * S, D).T.astype(NPBF))
    xvT = np.ascontiguousarray(v.reshape(B * S, D).T.astype(NPBF))
    wqT = np.ascontiguousarray(np.asarray(Wq, np.float32).T.astype(NPBF))
    wkT = np.ascontiguousarray(np.asarray(Wk, np.float32).T.astype(NPBF))
    wvT = np.ascontiguousarray(np.asarray(Wv, np.float32).T.astype(NPBF))
    bqt = np.ascontiguousarray(np.asarray(bq, np.float32).reshape(D // 128, 128).T)
    bkt = np.ascontiguousarray(np.asarray(bk, np.float32).reshape(D // 128, 128).T)
    bvt = np.ascontiguousarray(np.asarray(bv, np.float32).reshape(D // 128, 128).T)
    in1 = [
        {
            "xq": np.ascontiguousarray(xqT[:, c * RPC:(c + 1) * RPC]),
            "xk": np.ascontiguousarray(xkT[:, c * RPC:(c + 1) * RPC]),
            "xv": np.ascontiguousarray(xvT[:, c * RPC:(c + 1) * RPC]),
            "wq": wqT, "wk": wkT, "wv": wvT,
            "bq": bqt, "bk": bkt, "bv": bvt,
        }
        for c in cores
    ]
    r1 = run_bass_kernel_spmd(nc1, in1, core_ids=cores)
    QT = np.concatenate([r1.results[c]["qt"] for c in cores], axis=1)  # [D, B*S]
    KTm = np.concatenate([r1.results[c]["kt"] for c in cores], axis=1)
    VT = np.concatenate([r1.results[c]["vt"] for c in cores], axis=1)

    # ---------------- L2: attention (head-sharded) ----------------------
    m2 = mask.reshape(S, S)
    causal = bool(np.array_equal(m2 != 0, np.tril(np.ones((S, S), bool))))
    allones = bool((m2 != 0).all())
    use_causal = causal and not allones
    nc2 = _get(("attn", use_causal), _build_attn, use_causal)

    # V' per core: [B, 128, HPC, S//128, DK+1]
    Vh = VT.reshape(H, DK, B, S)                       # [h, d, b, s]
    in2 = []
    for c in cores:
        vp = np.empty((B, 128, HPC, S // 128, DK + 1), NPBF)
        for hh in range(HPC):
            h = HPC * c + hh
            # [d, b, s] -> [b, s, d] -> [b, t, p, d]
            vb = np.transpose(Vh[h], (1, 2, 0)).reshape(B, S // 128, 128, DK)
            vp[:, :, hh, :, :DK] = np.transpose(vb, (0, 2, 1, 3))
            vp[:, :, hh, :, DK] = 1.0
        m = {
            "qt": np.ascontiguousarray(QT[c * 128:(c + 1) * 128]),
            "kt": np.ascontiguousarray(KTm[c * 128:(c + 1) * 128]),
            "vp": vp,
        }
        if use_causal:
            m["m01"] = _strip_mask01()
        else:
            bias = np.where(m2 != 0, 0.0, NEG).astype(np.float32)
            if allones:
                bias[:] = 0.0
            # biasT[k, q] layout, tiled [S//128, 128, S]
            m["maskb"] = np.ascontiguousarray(bias.T.reshape(S // 128, 128, S))
        in2.append(m)
    r2 = run_bass_kernel_spmd(nc2, in2, core_ids=cores)

    # ---------------- normalize + L3: output projection -----------------
    UA = np.empty((D, B * S), np.float32)  # A^T, normalized
    for c in cores:
        u = r2.results[c]["u"]             # [B, HPC, DK+1, S]
        for hh in range(HPC):
            h = HPC * c + hh
            a = u[:, hh, :DK, :] / u[:, hh, DK:DK + 1, :]   # [B, DK, S]
            UA[h * DK:(h + 1) * DK] = np.transpose(a, (1, 0, 2)).reshape(DK, B * S)

    nc3 = _get("outproj", _build_outproj)
    UAb = UA.astype(NPBF)
    woT = np.ascontiguousarray(np.asarray(Wo, np.float32).T.astype(NPBF))
    in3 = [
        {"at": np.ascontiguousarray(UAb[:, c * RPC:(c + 1) * RPC]), "wo": woT}
        for c in cores
    ]
    r3 = run_bass_kernel_spmd(nc3, in3, core_ids=cores)
    y = np.concatenate([r3.results[c]["y"] for c in cores], axis=0)
    y = y + np.asarray(bo, np.float32)[None, :]
    return y.reshape(B, S, D)
